# revision 26
# baseline (speedup 1.0000x reference)
"""Additive (Bahdanau) attention on 8 Trainium2 NeuronCores.

Reference computation (per batch b):
    q = query @ Wq ; k = key @ Wk ; v = value @ Wv          [S, A]
    scores = tanh(q + k) @ Ws                               [S]
    w = softmax(scores)                                     [S]
    out  = (sum_s w[s] * v[s],  w)                          ([A], [S,1])

Kernel strategy:
  * Data-parallel over batch: B=16 -> 2 batches per core, no collectives.
  * Algebraic shortcut: sum_s w[s] * (value[s] @ Wv) == (sum_s w[s] * value[s]) @ Wv,
    so the value projection runs on one [1,D] row per batch instead of [S,D].
  * q+k projection fused into one K=1024 matmul: z^T = [Wq;Wk]^T @ [query;key]^T,
    computed in transposed orientation (host-side layout prep provides transposed
    operands), so the Ws contraction over A also runs on the TensorEngine.
  * The Ws weight column is replicated across 128 PE columns, so the scores
    matmul emits the score row broadcast to all 128 partitions at no extra cost.
    exp() of that broadcast feeds a VectorEngine fused multiply-reduce against a
    host-transposed value tensor (d on partitions): the weighted value sum costs
    zero TensorEngine work and its result lands pre-transposed for the final
    Wv projection.
  * Softmax without max-subtraction (scores are O(1) here; exp cannot overflow),
    normalization deferred to the very end (a per-partition scale on the outputs).
  * bf16 on-device storage/compute (fp32 PSUM/accumulator), halving HBM traffic.
  * Main-loop blocks run in pairs sharing each weight tile across two matmuls.
"""

import sys

import numpy as np

sys.path.insert(0, "/opt/trn_rl_repo")

import ml_dtypes  # noqa: E402

import concourse.bacc as bacc  # noqa: E402
import concourse.mybir as mybir  # noqa: E402
import concourse.tile as tile  # noqa: E402
from concourse import bass_utils  # noqa: E402

BF16 = mybir.dt.bfloat16
F32 = mybir.dt.float32
AF = mybir.ActivationFunctionType
ALU = mybir.AluOpType
NPBF16 = ml_dtypes.bfloat16

B, S, D, A = 16, 2048, 512, 512
NCORES = 8
BPC = B // NCORES          # batches per core
SL = BPC * S               # sequence positions per core
SB = 512                   # s-block (matmul moving dim)
NBLK = SL // SB            # s-blocks per core
BLKB = S // SB             # s-blocks per batch
KC = (2 * D) // 128        # contraction chunks for the fused q+k projection
AC = A // 128              # chunks of the attention feature dim
DC = D // 128              # chunks of the value feature dim

_CACHE: dict = {}


def _build():
    nc = bacc.Bacc("TRN2", target_bir_lowering=False, debug=False)

    qkT = nc.dram_tensor("qkT", [128, KC, SL], BF16, kind="ExternalInput")
    valT = nc.dram_tensor("valT", [128, DC, SL], BF16, kind="ExternalInput")
    wcat = nc.dram_tensor("wcat", [128, KC, A], BF16, kind="ExternalInput")
    wsr = nc.dram_tensor("wsr", [128, AC, 128], BF16, kind="ExternalInput")
    wvp = nc.dram_tensor("wvp", [128, DC, A], BF16, kind="ExternalInput")
    out_w = nc.dram_tensor("out_w", [BPC, S], F32, kind="ExternalOutput")
    out_ctx = nc.dram_tensor("out_ctx", [BPC, A], F32, kind="ExternalOutput")

    with tile.TileContext(nc) as tc:
        with (
            tc.tile_pool(name="singles", bufs=1) as singles,
            tc.tile_pool(name="qk", bufs=4) as qk_pool,
            tc.tile_pool(name="vv", bufs=4) as v_pool,
            tc.tile_pool(name="tt", bufs=3) as t_pool,
            tc.tile_pool(name="eb", bufs=2) as e_pool,
            tc.tile_pool(name="sm", bufs=4) as sm_pool,
            tc.tile_pool(name="ztps", bufs=3, space="PSUM") as zt_pool,
            tc.tile_pool(name="scps", bufs=2, space="PSUM") as sc_pool,
        ):
            # Startup: interleave weight-matrix halves with the first s-block's
            # input halves (separate tiles -> per-DMA dependencies) so the first
            # matmuls start after two half-DMAs.
            H = KC // 2
            sb_wc0 = singles.tile([128, H, A], BF16)
            qk_f0 = qk_pool.tile([128, H, SB], BF16, tag="qk_fh", name="qk_f0", bufs=2)
            sb_wc1 = singles.tile([128, H, A], BF16)
            qk_f1 = qk_pool.tile([128, H, SB], BF16, tag="qk_fh", name="qk_f1", bufs=2)
            nc.sync.dma_start(out=sb_wc0, in_=wcat.ap()[:, 0:H, :])
            nc.gpsimd.dma_start(out=qk_f0, in_=qkT.ap()[:, 0:H, 0:SB])
            nc.sync.dma_start(out=sb_wc1, in_=wcat.ap()[:, H:KC, :])
            nc.gpsimd.dma_start(out=qk_f1, in_=qkT.ap()[:, H:KC, 0:SB])

            def wc_lhsT(kc, a):
                t_ = sb_wc0 if kc < H else sb_wc1
                return t_[:, kc % H, a * 128:(a + 1) * 128]

            sb_wsr = singles.tile([128, AC, 128], BF16)
            nc.sync.dma_start(out=sb_wsr, in_=wsr.ap())
            v_first = v_pool.tile([128, DC, SB], BF16, name="v_t")
            nc.sync.dma_start(out=v_first, in_=valT.ap()[:, :, 0:SB])
            sb_wv = singles.tile([128, DC, A], BF16)
            nc.sync.dma_start(out=sb_wv, in_=wvp.ap())

            warm_w = singles.tile([128, 128], BF16)
            nc.vector.memset(warm_w, 0.0)
            warm_x = singles.tile([128, SB], BF16)
            nc.vector.memset(warm_x, 0.0)
            warm_ps = sc_pool.tile([128, SB], F32, tag="sc_ps", name="warm_ps")
            for _ in range(16):
                nc.tensor.matmul(
                    warm_ps, lhsT=warm_w, rhs=warm_x, start=True, stop=True
                )

            erow = singles.tile([1, BPC, S], F32)       # exp(scores) rows
            wrow = singles.tile([1, BPC, S], F32)       # normalized attn weights
            esum = singles.tile([128, BPC, BLKB + 1], F32)  # per-block exp sums
            rsum = singles.tile([1, BPC], F32)          # 1 / sum(exp) per batch
            # per-(block-in-batch, d-chunk) weighted-value partial sums
            acc = [
                singles.tile([128, BLKB + 1, DC], F32, name=f"acc{b_}")
                for b_ in range(BPC)
            ]

            def _exp_reduce_piece(b, jb, slot, sc_ps, v_t, lo, hi):
                # exp of scores[lo:hi] (broadcast), then the weighted value sum
                # for that span on the VectorEngine.
                n = hi - lo
                e_b = e_pool.tile([128, SB], BF16, name="e_b")
                nc.scalar.activation(
                    out=e_b[:, 0:n],
                    in_=sc_ps[:, lo:hi],
                    func=AF.Exp,
                    accum_out=esum[:, b, slot:slot + 1],
                )
                nc.vector.tensor_copy(
                    out=erow[0:1, b, jb * SB + lo: jb * SB + hi], in_=e_b[0:1, 0:n]
                )
                e_bc = e_b[:, 0:n].rearrange("p (c s) -> p c s", c=1).broadcast_to(
                    [128, DC, n]
                )
                wprod = sm_pool.tile(
                    [128, DC, SB], BF16, tag="wprod", name="wprod", bufs=2
                )
                nc.vector.tensor_mul(
                    wprod[:, :, 0:n],
                    v_t.rearrange("p c s -> p c s")[:, :, lo:hi],
                    e_bc,
                )
                nc.vector.reduce_sum(
                    out=acc[b][:, slot, :], in_=wprod[:, :, 0:n],
                    axis=mybir.AxisListType.X,
                )

            def emit_scores_exp_reduce(blk, tT, v_t):
                b = blk // BLKB
                jb = blk % BLKB
                # scores, broadcast to all 128 partitions by the replicated Ws
                sc_ps = sc_pool.tile([128, SB], F32, name="sc_ps")
                for a in range(AC):
                    nc.tensor.matmul(
                        sc_ps,
                        lhsT=sb_wsr[:, a, :],
                        rhs=tT[:, a, :],
                        start=(a == 0),
                        stop=(a == AC - 1),
                    )
                if jb == BLKB - 1:
                    # final block of the batch: two halves, so the tail's
                    # serial exp->DVE chain is half as long
                    _exp_reduce_piece(b, jb, jb, sc_ps, v_t, 0, SB // 2)
                    _exp_reduce_piece(b, jb, jb + 1, sc_ps, v_t, SB // 2, SB)
                else:
                    _exp_reduce_piece(b, jb, jb, sc_ps, v_t, 0, SB)

            def emit_batch_tail(b):
                # batch b complete: combine blocks, normalize, project
                tot = sm_pool.tile([1, 1], F32, tag="tot", name="tot")
                nc.vector.reduce_sum(
                    out=tot, in_=esum[0:1, b, :], axis=mybir.AxisListType.X
                )
                nc.vector.reciprocal(out=rsum[0:1, b:b + 1], in_=tot)

                # attention-weight row output first: ACT's 2us scale then runs
                # concurrently with the DVE context chain below
                nc.scalar.activation(
                    out=wrow[0:1, b, :],
                    in_=erow[0:1, b, :],
                    func=AF.Copy,
                    scale=rsum[0:1, b:b + 1],
                )
                nc.sync.dma_start(out=out_w.ap()[b:b + 1, :], in_=wrow[0:1, b, :])

                # sum the per-block partials -> ctx^T [128(d), DC], cast to bf16
                ctxTf = sm_pool.tile([128, DC], F32, tag="ctxTf", name="ctxTf")
                nc.vector.reduce_sum(
                    out=ctxTf,
                    in_=acc[b].rearrange("p j c -> p c j"),
                    axis=mybir.AxisListType.X,
                )
                ctxT = sm_pool.tile([128, DC], BF16, tag="ctxT", name="ctxT")
                nc.vector.tensor_copy(out=ctxT, in_=ctxTf)
                f_ps = sc_pool.tile([1, A], F32, tag="sc_ps", name="f_ps")
                for dc in range(DC):
                    nc.tensor.matmul(
                        f_ps,
                        lhsT=ctxT[:, dc:dc + 1],
                        rhs=sb_wv[:, dc, :],
                        start=(dc == 0),
                        stop=(dc == DC - 1),
                    )
                fctx = sm_pool.tile([1, A], F32, tag="fctx", name="fctx")
                nc.scalar.activation(
                    out=fctx, in_=f_ps, func=AF.Copy, scale=rsum[0:1, b:b + 1]
                )
                nc.sync.dma_start(out=out_ctx.ap()[b:b + 1, :], in_=fctx)

            # Blocks run in pairs so each weight tile is loaded once per two
            # matmuls (halving LDWEIGHTS pressure on the PE).
            for pr in range(NBLK // 2):
                blkA, blkB = 2 * pr, 2 * pr + 1

                if pr == 0:
                    qkA, vA = None, v_first  # qkA lives in qk_f0/qk_f1 halves
                else:
                    qkA = qk_pool.tile([128, KC, SB], BF16, name="qk_t")
                    nc.sync.dma_start(
                        out=qkA, in_=qkT.ap()[:, :, blkA * SB:(blkA + 1) * SB]
                    )
                    vA = v_pool.tile([128, DC, SB], BF16, name="v_t")
                    nc.sync.dma_start(
                        out=vA, in_=valT.ap()[:, :, blkA * SB:(blkA + 1) * SB]
                    )
                qkB = qk_pool.tile([128, KC, SB], BF16, name="qk_t")
                nc.sync.dma_start(
                    out=qkB, in_=qkT.ap()[:, :, blkB * SB:(blkB + 1) * SB]
                )
                vB = v_pool.tile([128, DC, SB], BF16, name="v_t")
                nc.sync.dma_start(
                    out=vB, in_=valT.ap()[:, :, blkB * SB:(blkB + 1) * SB]
                )

                def qkA_rhs(kc):
                    if pr == 0:
                        t_ = qk_f0 if kc < H else qk_f1
                        return t_[:, kc % H, :]
                    return qkA[:, kc, :]

                # z^T[a, s] for both blocks; a-chunks in pairs sharing one PSUM
                # tile (adjacent banks) so one tanh covers both — ACT's
                # ~350-cycle per-op bubble is paid half as often.
                tTA = t_pool.tile([128, AC, SB], BF16, name="tT")
                tTB = t_pool.tile([128, AC, SB], BF16, name="tT")
                if pr == 0:
                    # Sequential blocks at startup: the first matmuls gate only
                    # on the first half-DMAs; block B's input lands while A runs.
                    for tT_, rhs_of in (
                        (tTA, qkA_rhs),
                        (tTB, lambda kc: qkB[:, kc, :]),
                    ):
                        for ap_ in range(AC // 2):
                            z_ps = zt_pool.tile([128, 2, SB], F32, name="z_ps")
                            for half in range(2):
                                a = 2 * ap_ + half
                                for kc in range(KC):
                                    nc.tensor.matmul(
                                        z_ps[:, half, :],
                                        lhsT=wc_lhsT(kc, a),
                                        rhs=rhs_of(kc),
                                        start=(kc == 0),
                                        stop=(kc == KC - 1),
                                    )
                            nc.scalar.activation(
                                out=tT_[:, 2 * ap_:2 * ap_ + 2, :], in_=z_ps,
                                func=AF.Tanh,
                            )
                else:
                    for ap_ in range(AC // 2):
                        zA = zt_pool.tile([128, 2, SB], F32, name="z_ps")
                        zB = zt_pool.tile([128, 2, SB], F32, name="z_ps")
                        for half in range(2):
                            a = 2 * ap_ + half
                            for kc in range(KC):
                                lhsT = wc_lhsT(kc, a)
                                nc.tensor.matmul(
                                    zA[:, half, :],
                                    lhsT=lhsT,
                                    rhs=qkA_rhs(kc),
                                    start=(kc == 0),
                                    stop=(kc == KC - 1),
                                )
                                nc.tensor.matmul(
                                    zB[:, half, :],
                                    lhsT=lhsT,
                                    rhs=qkB[:, kc, :],
                                    start=(kc == 0),
                                    stop=(kc == KC - 1),
                                )
                        nc.scalar.activation(
                            out=tTA[:, 2 * ap_:2 * ap_ + 2, :], in_=zA, func=AF.Tanh
                        )
                        nc.scalar.activation(
                            out=tTB[:, 2 * ap_:2 * ap_ + 2, :], in_=zB, func=AF.Tanh
                        )

                emit_scores_exp_reduce(blkA, tTA, vA)
                emit_scores_exp_reduce(blkB, tTB, vB)
                if blkB % BLKB == BLKB - 1:
                    emit_batch_tail(blkB // BLKB)

    nc.compile()
    return nc


def _get_nc():
    if "nc" not in _CACHE:
        _CACHE["nc"] = _build()
    return _CACHE["nc"]


def _prep_core(q2, k2, v2):
    """Host-side layout prep for one core's shard (free: not on-device time)."""
    xcatT = np.concatenate([q2.T, k2.T], 0)  # [2D, SL]
    qkT = np.ascontiguousarray(
        xcatT.reshape(KC, 128, SL).transpose(1, 0, 2)
    ).astype(NPBF16)
    valT = np.ascontiguousarray(
        v2.T.reshape(DC, 128, SL).transpose(1, 0, 2)
    ).astype(NPBF16)
    return qkT, valT


def _prep_weights(Wq, Wk, Wv, Ws):
    Wcat = np.concatenate([Wq, Wk], 0)  # [2D, A]
    wcat_h = np.ascontiguousarray(
        Wcat.reshape(KC, 128, A).transpose(1, 0, 2)
    ).astype(NPBF16)
    # Ws replicated across 128 PE columns: the scores matmul then broadcasts
    # the score row to every output partition.
    ws_pa = Ws[:, 0].reshape(AC, 128).transpose(1, 0)  # [128, AC]
    wsr_h = np.ascontiguousarray(
        np.repeat(ws_pa[:, :, None], 128, axis=2)
    ).astype(NPBF16)
    wvp_h = np.ascontiguousarray(
        Wv.reshape(DC, 128, A).transpose(1, 0, 2)
    ).astype(NPBF16)
    return wcat_h, wsr_h, wvp_h


def build_in_maps(query, key_, value, Wq, Wk, Wv, Ws):
    query = np.asarray(query, dtype=np.float32)
    key_ = np.asarray(key_, dtype=np.float32)
    value = np.asarray(value, dtype=np.float32)
    wcat_h, wsr_h, wvp_h = _prep_weights(
        np.asarray(Wq, dtype=np.float32),
        np.asarray(Wk, dtype=np.float32),
        np.asarray(Wv, dtype=np.float32),
        np.asarray(Ws, dtype=np.float32),
    )
    in_maps = []
    for c in range(NCORES):
        q2 = query[c * BPC:(c + 1) * BPC].reshape(SL, D)
        k2 = key_[c * BPC:(c + 1) * BPC].reshape(SL, D)
        v2 = value[c * BPC:(c + 1) * BPC].reshape(SL, D)
        qkT_h, valT_h = _prep_core(q2, k2, v2)
        in_maps.append(
            {"qkT": qkT_h, "valT": valT_h, "wcat": wcat_h, "wsr": wsr_h,
             "wvp": wvp_h}
        )
    return in_maps


def kernel(query, key_, value, Wq, Wk, Wv, Ws):
    nc = _get_nc()
    in_maps = build_in_maps(query, key_, value, Wq, Wk, Wv, Ws)

    res = bass_utils.run_bass_kernel_spmd(
        nc, in_maps, core_ids=list(range(NCORES))
    )

    ctx = np.concatenate(
        [np.asarray(r["out_ctx"], dtype=np.float32) for r in res.results], 0
    )
    attw = np.concatenate(
        [np.asarray(r["out_w"], dtype=np.float32) for r in res.results], 0
    )[..., None]
    return ctx, attw


# revision 42
# speedup vs baseline: 1.0254x; 1.0254x over previous
"""Additive (Bahdanau) attention on 8 Trainium2 NeuronCores.

Reference computation (per batch b):
    q = query @ Wq ; k = key @ Wk ; v = value @ Wv          [S, A]
    scores = tanh(q + k) @ Ws                               [S]
    w = softmax(scores)                                     [S]
    out  = (sum_s w[s] * v[s],  w)                          ([A], [S,1])

Kernel strategy:
  * Data-parallel over batch: B=16 -> 2 batches per core, no collectives.
  * Algebraic shortcut: sum_s w[s] * (value[s] @ Wv) == (sum_s w[s] * value[s]) @ Wv,
    so the value projection runs on one [1,D] row per batch instead of [S,D].
  * q+k projection fused into one K=1024 matmul: z^T = [Wq;Wk]^T @ [query;key]^T,
    computed in transposed orientation (host-side layout prep provides transposed
    operands), so the Ws contraction over A also runs on the TensorEngine.
  * The Ws weight column is replicated across 128 PE columns, so the scores
    matmul emits the score row broadcast to all 128 partitions at no extra cost.
    exp() of that broadcast feeds a VectorEngine fused multiply-reduce against a
    host-transposed value tensor (d on partitions): the weighted value sum costs
    zero TensorEngine work and its result lands pre-transposed for the final
    Wv projection.
  * Softmax without max-subtraction (scores are O(1) here; exp cannot overflow),
    normalization deferred to the very end (a per-partition scale on the outputs).
  * bf16 on-device storage/compute (fp32 PSUM/accumulator), halving HBM traffic.
  * Main-loop blocks run in pairs sharing each weight tile across two matmuls.
"""

import sys

import numpy as np

sys.path.insert(0, "/opt/trn_rl_repo")

import ml_dtypes  # noqa: E402

import concourse.bacc as bacc  # noqa: E402
import concourse.mybir as mybir  # noqa: E402
import concourse.tile as tile  # noqa: E402
from concourse import bass_utils  # noqa: E402

BF16 = mybir.dt.bfloat16
F32 = mybir.dt.float32
AF = mybir.ActivationFunctionType
ALU = mybir.AluOpType
NPBF16 = ml_dtypes.bfloat16

B, S, D, A = 16, 2048, 512, 512
NCORES = 8
BPC = B // NCORES          # batches per core
SL = BPC * S               # sequence positions per core
SB = 512                   # s-block (matmul moving dim)
NBLK = SL // SB            # s-blocks per core
BLKB = S // SB             # s-blocks per batch
KC = (2 * D) // 128        # contraction chunks for the fused q+k projection
AC = A // 128              # chunks of the attention feature dim
DC = D // 128              # chunks of the value feature dim

_CACHE: dict = {}


def _build():
    nc = bacc.Bacc("TRN2", target_bir_lowering=False, debug=False)

    qkT = nc.dram_tensor("qkT", [128, KC, SL], BF16, kind="ExternalInput")
    valT = nc.dram_tensor("valT", [128, DC, SL], BF16, kind="ExternalInput")
    wcat = nc.dram_tensor("wcat", [128, KC, A], BF16, kind="ExternalInput")
    wsr = nc.dram_tensor("wsr", [128, AC, 128], BF16, kind="ExternalInput")
    wvp = nc.dram_tensor("wvp", [128, DC, A], BF16, kind="ExternalInput")
    out_w = nc.dram_tensor("out_w", [BPC, S], F32, kind="ExternalOutput")
    out_ctx = nc.dram_tensor("out_ctx", [BPC, A], F32, kind="ExternalOutput")

    with tile.TileContext(nc) as tc:
        with (
            tc.tile_pool(name="singles", bufs=1) as singles,
            tc.tile_pool(name="qk", bufs=4) as qk_pool,
            tc.tile_pool(name="vv", bufs=4) as v_pool,
            tc.tile_pool(name="tt", bufs=3) as t_pool,
            tc.tile_pool(name="eb", bufs=2) as e_pool,
            tc.tile_pool(name="sm", bufs=4) as sm_pool,
            tc.tile_pool(name="ztps", bufs=3, space="PSUM") as zt_pool,
            tc.tile_pool(name="scps", bufs=2, space="PSUM") as sc_pool,
        ):
            # Startup: interleave weight-matrix halves with the first s-block's
            # input halves (separate tiles -> per-DMA dependencies) so the first
            # matmuls start after two half-DMAs.
            Q = KC // 2
            wc_q = [
                singles.tile([128, Q, A], BF16, name=f"wc_q{i}") for i in range(2)
            ]
            qk_q = [
                qk_pool.tile([128, Q, SB], BF16, tag="qk_fh", name=f"qk_q{i}",
                             bufs=2)
                for i in range(2)
            ]
            for i in range(2):
                nc.sync.dma_start(
                    out=wc_q[i], in_=wcat.ap()[:, i * Q:(i + 1) * Q, :]
                )
                nc.gpsimd.dma_start(
                    out=qk_q[i], in_=qkT.ap()[:, i * Q:(i + 1) * Q, 0:SB]
                )

            def wc_lhsT(kc, a):
                return wc_q[kc // Q][:, kc % Q, a * 128:(a + 1) * 128]

            sb_wsr = singles.tile([128, AC, 128], BF16)
            nc.sync.dma_start(out=sb_wsr, in_=wsr.ap())
            v_first = v_pool.tile([128, DC, SB], BF16, name="v_t")
            nc.sync.dma_start(out=v_first, in_=valT.ap()[:, :, 0:SB])
            sb_wv = singles.tile([128, DC, A], BF16)
            nc.sync.dma_start(out=sb_wv, in_=wvp.ap())

            warm_w = singles.tile([128, 128], BF16)
            nc.vector.memset(warm_w, 0.0)
            warm_x = singles.tile([128, SB], BF16)
            nc.vector.memset(warm_x, 0.0)
            warm_ps = sc_pool.tile([128, SB], F32, tag="sc_ps", name="warm_ps")
            for _ in range(28):
                nc.tensor.matmul(
                    warm_ps, lhsT=warm_w, rhs=warm_x, start=True, stop=True
                )

            erow = singles.tile([1, BPC, S], F32)       # exp(scores) rows
            wrow = singles.tile([1, BPC, S], F32)       # normalized attn weights
            esum = singles.tile([128, BPC, BLKB + 3], F32)  # per-block exp sums
            nc.vector.memset(esum, 0.0)
            rsum = singles.tile([1, BPC], F32)          # 1 / sum(exp) per batch
            # per-(block-in-batch, d-chunk) weighted-value partial sums
            acc = [
                singles.tile([128, BLKB + 3, DC], F32, name=f"acc{b_}")
                for b_ in range(BPC)
            ]
            for b_ in range(BPC):
                nc.vector.memset(acc[b_], 0.0)

            def _exp_reduce_piece(b, jb, slot, sc_ps, v_t, lo, hi, eng=None):
                # exp of scores[lo:hi] (broadcast), then the weighted value sum
                # for that span on the Vector (or GpSimd) engine.
                eng = eng or nc.vector
                n = hi - lo
                e_b = e_pool.tile([128, SB], BF16, name="e_b")
                nc.scalar.activation(
                    out=e_b[:, 0:n],
                    in_=sc_ps[:, lo:hi],
                    func=AF.Exp,
                    accum_out=esum[:, b, slot:slot + 1],
                )
                nc.vector.tensor_copy(
                    out=erow[0:1, b, jb * SB + lo: jb * SB + hi], in_=e_b[0:1, 0:n]
                )
                e_bc = e_b[:, 0:n].rearrange("p (c s) -> p c s", c=1).broadcast_to(
                    [128, DC, n]
                )
                wprod = sm_pool.tile(
                    [128, DC, SB], BF16, tag="wprod", name="wprod", bufs=3
                )
                eng.tensor_mul(
                    wprod[:, :, 0:n],
                    v_t.rearrange("p c s -> p c s")[:, :, lo:hi],
                    e_bc,
                )
                nc.vector.reduce_sum(
                    out=acc[b][:, slot, :], in_=wprod[:, :, 0:n],
                    axis=mybir.AxisListType.X,
                )
                return e_b

            def emit_scores_exp_reduce(blk, tT, v_t):
                b = blk // BLKB
                jb = blk % BLKB
                # scores, broadcast to all 128 partitions by the replicated Ws
                sc_ps = sc_pool.tile([128, SB], F32, name="sc_ps")
                for a in range(AC):
                    nc.tensor.matmul(
                        sc_ps,
                        lhsT=sb_wsr[:, a, :],
                        rhs=tT[:, a, :],
                        start=(a == 0),
                        stop=(a == AC - 1),
                    )
                if jb == BLKB - 1 and b == BPC - 1:
                    # very last block: four quarters pipeline ACT/DVE tightly;
                    # after each quarter, a couple of throwaway matmuls chained
                    # on its exp output keep the PE clock-warm through the tail
                    qn = SB // 4
                    warm2 = sc_pool.tile([128, SB], F32, tag="sc_ps", name="warm2")
                    for i4 in range(4):
                        eq = _exp_reduce_piece(
                            b, jb, jb + i4, sc_ps, v_t,
                            i4 * qn, (i4 + 1) * qn,
                        )
                        for _ in range(2):
                            nc.tensor.matmul(
                                warm2[:, 0:qn], lhsT=warm_w, rhs=eq[:, 0:qn],
                                start=True, stop=True,
                            )
                elif jb == BLKB - 1:
                    # final block of the batch: two halves, so the tail's
                    # serial exp->DVE chain is half as long
                    _exp_reduce_piece(b, jb, jb, sc_ps, v_t, 0, SB // 2)
                    _exp_reduce_piece(b, jb, jb + 1, sc_ps, v_t, SB // 2, SB)
                else:
                    _exp_reduce_piece(b, jb, jb, sc_ps, v_t, 0, SB)

            def emit_batch_tail(b):
                # batch b complete: combine blocks, normalize, project
                tot = sm_pool.tile([1, 1], F32, tag="tot", name="tot")
                nc.vector.reduce_sum(
                    out=tot, in_=esum[0:1, b, :], axis=mybir.AxisListType.X
                )
                nc.vector.reciprocal(out=rsum[0:1, b:b + 1], in_=tot)

                # attention-weight row output first: ACT's 2us scale then runs
                # concurrently with the DVE context chain below
                nc.scalar.activation(
                    out=wrow[0:1, b, :],
                    in_=erow[0:1, b, :],
                    func=AF.Copy,
                    scale=rsum[0:1, b:b + 1],
                )
                nc.sync.dma_start(out=out_w.ap()[b:b + 1, :], in_=wrow[0:1, b, :])

                # sum the per-block partials -> ctx^T [128(d), DC], cast to bf16
                ctxTf = sm_pool.tile([128, DC], F32, tag="ctxTf", name="ctxTf")
                nc.vector.reduce_sum(
                    out=ctxTf,
                    in_=acc[b].rearrange("p j c -> p c j"),
                    axis=mybir.AxisListType.X,
                )
                ctxT = sm_pool.tile([128, DC], BF16, tag="ctxT", name="ctxT")
                nc.vector.tensor_copy(out=ctxT, in_=ctxTf)
                f_ps = sc_pool.tile([1, A], F32, tag="sc_ps", name="f_ps")
                for dc in range(DC):
                    nc.tensor.matmul(
                        f_ps,
                        lhsT=ctxT[:, dc:dc + 1],
                        rhs=sb_wv[:, dc, :],
                        start=(dc == 0),
                        stop=(dc == DC - 1),
                    )
                fctx = sm_pool.tile([1, A], F32, tag="fctx", name="fctx")
                nc.vector.tensor_scalar_mul(fctx, f_ps, rsum[0:1, b:b + 1])
                nc.sync.dma_start(out=out_ctx.ap()[b:b + 1, :], in_=fctx)

            # Sequential blocks: each block's exp/DVE chain overlaps the next
            # block's projection matmuls, so only the final block's chain is
            # exposed at the tail.
            for blk in range(NBLK):
                if blk == 0:
                    qk_t, v_t = None, v_first  # qk lives in qk_f0/qk_f1 halves
                else:
                    qk_t = qk_pool.tile([128, KC, SB], BF16, name="qk_t")
                    nc.sync.dma_start(
                        out=qk_t, in_=qkT.ap()[:, :, blk * SB:(blk + 1) * SB]
                    )
                    v_t = v_pool.tile([128, DC, SB], BF16, name="v_t")
                    nc.sync.dma_start(
                        out=v_t, in_=valT.ap()[:, :, blk * SB:(blk + 1) * SB]
                    )
                if blk == 0:
                    def rhs_of(kc):
                        return qk_q[kc // Q][:, kc % Q, :]
                    v_t = v_first
                else:
                    def rhs_of(kc, qk_t=qk_t):
                        return qk_t[:, kc, :]

                tT = t_pool.tile([128, AC, SB], BF16, name="tT")
                for ap_ in range(AC // 2):
                    z_ps = zt_pool.tile([128, 2, SB], F32, name="z_ps")
                    for half in range(2):
                        a = 2 * ap_ + half
                        for kc in range(KC):
                            nc.tensor.matmul(
                                z_ps[:, half, :],
                                lhsT=wc_lhsT(kc, a),
                                rhs=rhs_of(kc),
                                start=(kc == 0),
                                stop=(kc == KC - 1),
                            )
                    nc.scalar.activation(
                        out=tT[:, 2 * ap_:2 * ap_ + 2, :], in_=z_ps, func=AF.Tanh
                    )

                emit_scores_exp_reduce(blk, tT, v_t)
                if blk % BLKB == BLKB - 1:
                    emit_batch_tail(blk // BLKB)

    nc.compile()
    return nc


def _get_nc():
    if "nc" not in _CACHE:
        _CACHE["nc"] = _build()
    return _CACHE["nc"]


def _prep_core(q2, k2, v2):
    """Host-side layout prep for one core's shard (free: not on-device time)."""
    xcatT = np.concatenate([q2.T, k2.T], 0)  # [2D, SL]
    qkT = np.ascontiguousarray(
        xcatT.reshape(KC, 128, SL).transpose(1, 0, 2)
    ).astype(NPBF16)
    valT = np.ascontiguousarray(
        v2.T.reshape(DC, 128, SL).transpose(1, 0, 2)
    ).astype(NPBF16)
    return qkT, valT


def _prep_weights(Wq, Wk, Wv, Ws):
    Wcat = np.concatenate([Wq, Wk], 0)  # [2D, A]
    wcat_h = np.ascontiguousarray(
        Wcat.reshape(KC, 128, A).transpose(1, 0, 2)
    ).astype(NPBF16)
    # Ws replicated across 128 PE columns: the scores matmul then broadcasts
    # the score row to every output partition.
    ws_pa = Ws[:, 0].reshape(AC, 128).transpose(1, 0)  # [128, AC]
    wsr_h = np.ascontiguousarray(
        np.repeat(ws_pa[:, :, None], 128, axis=2)
    ).astype(NPBF16)
    wvp_h = np.ascontiguousarray(
        Wv.reshape(DC, 128, A).transpose(1, 0, 2)
    ).astype(NPBF16)
    return wcat_h, wsr_h, wvp_h


def build_in_maps(query, key_, value, Wq, Wk, Wv, Ws):
    query = np.asarray(query, dtype=np.float32)
    key_ = np.asarray(key_, dtype=np.float32)
    value = np.asarray(value, dtype=np.float32)
    wcat_h, wsr_h, wvp_h = _prep_weights(
        np.asarray(Wq, dtype=np.float32),
        np.asarray(Wk, dtype=np.float32),
        np.asarray(Wv, dtype=np.float32),
        np.asarray(Ws, dtype=np.float32),
    )
    in_maps = []
    for c in range(NCORES):
        q2 = query[c * BPC:(c + 1) * BPC].reshape(SL, D)
        k2 = key_[c * BPC:(c + 1) * BPC].reshape(SL, D)
        v2 = value[c * BPC:(c + 1) * BPC].reshape(SL, D)
        qkT_h, valT_h = _prep_core(q2, k2, v2)
        in_maps.append(
            {"qkT": qkT_h, "valT": valT_h, "wcat": wcat_h, "wsr": wsr_h,
             "wvp": wvp_h}
        )
    return in_maps


def kernel(query, key_, value, Wq, Wk, Wv, Ws):
    nc = _get_nc()
    in_maps = build_in_maps(query, key_, value, Wq, Wk, Wv, Ws)

    res = bass_utils.run_bass_kernel_spmd(
        nc, in_maps, core_ids=list(range(NCORES))
    )

    ctx = np.concatenate(
        [np.asarray(r["out_ctx"], dtype=np.float32) for r in res.results], 0
    )
    attw = np.concatenate(
        [np.asarray(r["out_w"], dtype=np.float32) for r in res.results], 0
    )[..., None]
    return ctx, attw


# revision 43
# speedup vs baseline: 1.1876x; 1.1581x over previous
"""Additive (Bahdanau) attention on 8 Trainium2 NeuronCores.

Reference computation (per batch b):
    q = query @ Wq ; k = key @ Wk ; v = value @ Wv          [S, A]
    scores = tanh(q + k) @ Ws                               [S]
    w = softmax(scores)                                     [S]
    out  = (sum_s w[s] * v[s],  w)                          ([A], [S,1])

Kernel strategy:
  * Data-parallel over batch: B=16 -> 2 batches per core, no collectives.
  * Algebraic shortcut: sum_s w[s] * (value[s] @ Wv) == (sum_s w[s] * value[s]) @ Wv,
    so the value projection runs on one [1,D] row per batch instead of [S,D].
  * q+k projection fused into one K=1024 matmul: z^T = [Wq;Wk]^T @ [query;key]^T,
    computed in transposed orientation (host-side layout prep provides transposed
    operands), so the Ws contraction over A also runs on the TensorEngine.
  * The Ws weight column is replicated across 128 PE columns, so the scores
    matmul emits the score row broadcast to all 128 partitions at no extra cost.
    exp() of that broadcast feeds a VectorEngine fused multiply-reduce against a
    host-transposed value tensor (d on partitions): the weighted value sum costs
    zero TensorEngine work and its result lands pre-transposed for the final
    Wv projection.
  * Softmax without max-subtraction (scores are O(1) here; exp cannot overflow),
    normalization deferred to the very end (a per-partition scale on the outputs).
  * bf16 on-device storage/compute (fp32 PSUM/accumulator), halving HBM traffic.
  * Main-loop blocks run in pairs sharing each weight tile across two matmuls.
"""

import sys

import numpy as np

sys.path.insert(0, "/opt/trn_rl_repo")

import ml_dtypes  # noqa: E402

import concourse.bacc as bacc  # noqa: E402
import concourse.mybir as mybir  # noqa: E402
import concourse.tile as tile  # noqa: E402
from concourse import bass_utils  # noqa: E402

BF16 = mybir.dt.bfloat16
F32 = mybir.dt.float32
AF = mybir.ActivationFunctionType
ALU = mybir.AluOpType
NPBF16 = ml_dtypes.bfloat16

B, S, D, A = 16, 2048, 512, 512
NCORES = 8
BPC = B // NCORES          # batches per core
SL = BPC * S               # sequence positions per core
SB = 512                   # s-block (matmul moving dim)
NBLK = SL // SB            # s-blocks per core
BLKB = S // SB             # s-blocks per batch
KC = (2 * D) // 128        # contraction chunks for the fused q+k projection
AC = A // 128              # chunks of the attention feature dim
DC = D // 128              # chunks of the value feature dim

_CACHE: dict = {}


def _build():
    nc = bacc.Bacc("TRN2", target_bir_lowering=False, debug=False)

    qkT = nc.dram_tensor("qkT", [128, KC, SL], BF16, kind="ExternalInput")
    valT = nc.dram_tensor("valT", [128, DC, SL], BF16, kind="ExternalInput")
    wcat = nc.dram_tensor("wcat", [128, KC, A], BF16, kind="ExternalInput")
    wsr = nc.dram_tensor("wsr", [128, AC, 128], BF16, kind="ExternalInput")
    wvp = nc.dram_tensor("wvp", [128, DC, A], BF16, kind="ExternalInput")
    out_w = nc.dram_tensor("out_w", [BPC, S], F32, kind="ExternalOutput")
    out_ctx = nc.dram_tensor("out_ctx", [BPC, A], F32, kind="ExternalOutput")

    with tile.TileContext(nc) as tc:
        with (
            tc.tile_pool(name="singles", bufs=1) as singles,
            tc.tile_pool(name="qk", bufs=4) as qk_pool,
            tc.tile_pool(name="vv", bufs=4) as v_pool,
            tc.tile_pool(name="tt", bufs=3) as t_pool,
            tc.tile_pool(name="eb", bufs=2) as e_pool,
            tc.tile_pool(name="sm", bufs=4) as sm_pool,
            tc.tile_pool(name="ztps", bufs=3, space="PSUM") as zt_pool,
            tc.tile_pool(name="scps", bufs=2, space="PSUM") as sc_pool,
        ):
            # Startup: interleave weight-matrix halves with the first s-block's
            # input halves (separate tiles -> per-DMA dependencies) so the first
            # matmuls start after two half-DMAs.
            Q = KC // 2
            wc_q = [
                singles.tile([128, Q, A], BF16, name=f"wc_q{i}") for i in range(2)
            ]
            qk_q = [
                qk_pool.tile([128, Q, SB], BF16, tag="qk_fh", name=f"qk_q{i}",
                             bufs=2)
                for i in range(2)
            ]
            for i in range(2):
                nc.sync.dma_start(
                    out=wc_q[i], in_=wcat.ap()[:, i * Q:(i + 1) * Q, :]
                )
                nc.gpsimd.dma_start(
                    out=qk_q[i], in_=qkT.ap()[:, i * Q:(i + 1) * Q, 0:SB]
                )

            def wc_lhsT(kc, a):
                return wc_q[kc // Q][:, kc % Q, a * 128:(a + 1) * 128]

            sb_wsr = singles.tile([128, AC, 128], BF16)
            nc.sync.dma_start(out=sb_wsr, in_=wsr.ap())
            v_first = v_pool.tile([128, DC, SB], BF16, name="v_t")
            nc.sync.dma_start(out=v_first, in_=valT.ap()[:, :, 0:SB])
            sb_wv = singles.tile([128, DC, A], BF16)
            nc.sync.dma_start(out=sb_wv, in_=wvp.ap())

            warm_w = singles.tile([128, 128], BF16)
            nc.vector.memset(warm_w, 0.0)
            warm_x = singles.tile([128, SB], BF16)
            nc.vector.memset(warm_x, 0.0)
            warm_ps = sc_pool.tile([128, SB], F32, tag="sc_ps", name="warm_ps")
            for _ in range(28):
                nc.tensor.matmul(
                    warm_ps, lhsT=warm_w, rhs=warm_x, start=True, stop=True
                )

            erow = singles.tile([1, BPC, S], F32)       # exp(scores) rows
            wrow = singles.tile([1, BPC, S], F32)       # normalized attn weights
            esum = singles.tile([128, BPC, BLKB + 3], F32)  # per-block exp sums
            nc.vector.memset(esum, 0.0)
            rsum = singles.tile([1, BPC], F32)          # 1 / sum(exp) per batch
            # per-(block-in-batch, d-chunk) weighted-value partial sums
            acc = [
                singles.tile([128, BLKB + 3, DC], F32, name=f"acc{b_}")
                for b_ in range(BPC)
            ]
            for b_ in range(BPC):
                nc.vector.memset(acc[b_], 0.0)

            tail_fps = [None]

            def _exp_reduce_piece(b, jb, slot, sc_ps, v_t, lo, hi, eng=None):
                # exp of scores[lo:hi] (broadcast), then the weighted value sum
                # for that span on the Vector (or GpSimd) engine.
                eng = eng or nc.vector
                n = hi - lo
                e_b = e_pool.tile([128, SB], BF16, name="e_b")
                nc.scalar.activation(
                    out=e_b[:, 0:n],
                    in_=sc_ps[:, lo:hi],
                    func=AF.Exp,
                    accum_out=esum[:, b, slot:slot + 1],
                )
                nc.vector.tensor_copy(
                    out=erow[0:1, b, jb * SB + lo: jb * SB + hi], in_=e_b[0:1, 0:n]
                )
                e_bc = e_b[:, 0:n].rearrange("p (c s) -> p c s", c=1).broadcast_to(
                    [128, DC, n]
                )
                wprod = sm_pool.tile(
                    [128, DC, SB], BF16, tag="wprod", name="wprod", bufs=3
                )
                eng.tensor_mul(
                    wprod[:, :, 0:n],
                    v_t.rearrange("p c s -> p c s")[:, :, lo:hi],
                    e_bc,
                )
                nc.vector.reduce_sum(
                    out=acc[b][:, slot, :], in_=wprod[:, :, 0:n],
                    axis=mybir.AxisListType.X,
                )
                return e_b

            def emit_scores_exp_reduce(blk, tT, v_t):
                b = blk // BLKB
                jb = blk % BLKB
                # scores, broadcast to all 128 partitions by the replicated Ws
                sc_ps = sc_pool.tile([128, SB], F32, name="sc_ps")
                for a in range(AC):
                    nc.tensor.matmul(
                        sc_ps,
                        lhsT=sb_wsr[:, a, :],
                        rhs=tT[:, a, :],
                        start=(a == 0),
                        stop=(a == AC - 1),
                    )
                if jb == BLKB - 1 and b == BPC - 1:
                    # very last block: four quarters pipeline ACT/DVE tightly,
                    # and every accumulator slot is projected through Wv as soon
                    # as it lands — the projection matmuls fill the PE's tail
                    # window with real (clock-warming) work, leaving only the
                    # final quarter's four matmuls serial.
                    qn = SB // 4
                    ctxTaf = sm_pool.tile([128, DC], F32, tag="ctxTf",
                                          name="ctxTaf")
                    nc.vector.reduce_sum(
                        out=ctxTaf,
                        in_=acc[b][:, 0:BLKB - 1, :].rearrange("p j c -> p c j"),
                        axis=mybir.AxisListType.X,
                    )
                    ctxTa = sm_pool.tile([128, DC], BF16, tag="ctxT",
                                         name="ctxTa")
                    nc.vector.tensor_copy(out=ctxTa, in_=ctxTaf)
                    fps2 = sc_pool.tile([1, A], F32, tag="sc_ps", name="fps2")
                    for dc in range(DC):
                        nc.tensor.matmul(
                            fps2,
                            lhsT=ctxTa[:, dc:dc + 1],
                            rhs=sb_wv[:, dc, :],
                            start=(dc == 0),
                            stop=False,
                        )
                    for i4 in range(4):
                        slot = jb + i4
                        _exp_reduce_piece(
                            b, jb, slot, sc_ps, v_t, i4 * qn, (i4 + 1) * qn
                        )
                        ctxq = sm_pool.tile([128, DC], BF16, tag="ctxT",
                                            name="ctxq")
                        nc.vector.tensor_copy(out=ctxq, in_=acc[b][:, slot, :])
                        for dc in range(DC):
                            nc.tensor.matmul(
                                fps2,
                                lhsT=ctxq[:, dc:dc + 1],
                                rhs=sb_wv[:, dc, :],
                                start=False,
                                stop=(i4 == 3 and dc == DC - 1),
                            )
                    tail_fps[0] = fps2
                elif jb == BLKB - 1:
                    # final block of the batch: two halves, so the tail's
                    # serial exp->DVE chain is half as long
                    _exp_reduce_piece(b, jb, jb, sc_ps, v_t, 0, SB // 2)
                    _exp_reduce_piece(b, jb, jb + 1, sc_ps, v_t, SB // 2, SB)
                else:
                    _exp_reduce_piece(b, jb, jb, sc_ps, v_t, 0, SB)

            def emit_batch_tail(b):
                # batch b complete: combine blocks, normalize, project
                tot = sm_pool.tile([1, 1], F32, tag="tot", name="tot")
                nc.vector.reduce_sum(
                    out=tot, in_=esum[0:1, b, :], axis=mybir.AxisListType.X
                )
                nc.vector.reciprocal(out=rsum[0:1, b:b + 1], in_=tot)

                # attention-weight row output first: ACT's 2us scale then runs
                # concurrently with the DVE context chain below
                nc.scalar.activation(
                    out=wrow[0:1, b, :],
                    in_=erow[0:1, b, :],
                    func=AF.Copy,
                    scale=rsum[0:1, b:b + 1],
                )
                nc.sync.dma_start(out=out_w.ap()[b:b + 1, :], in_=wrow[0:1, b, :])

                if b == BPC - 1 and tail_fps[0] is not None:
                    f_ps = tail_fps[0]
                else:
                    # sum the per-block partials -> ctx^T, cast, project
                    ctxTf = sm_pool.tile([128, DC], F32, tag="ctxTf", name="ctxTf")
                    nc.vector.reduce_sum(
                        out=ctxTf,
                        in_=acc[b].rearrange("p j c -> p c j"),
                        axis=mybir.AxisListType.X,
                    )
                    ctxT = sm_pool.tile([128, DC], BF16, tag="ctxT", name="ctxT")
                    nc.vector.tensor_copy(out=ctxT, in_=ctxTf)
                    f_ps = sc_pool.tile([1, A], F32, tag="sc_ps", name="f_ps")
                    for dc in range(DC):
                        nc.tensor.matmul(
                            f_ps,
                            lhsT=ctxT[:, dc:dc + 1],
                            rhs=sb_wv[:, dc, :],
                            start=(dc == 0),
                            stop=(dc == DC - 1),
                        )
                fctx = sm_pool.tile([1, A], F32, tag="fctx", name="fctx")
                nc.vector.tensor_scalar_mul(fctx, f_ps, rsum[0:1, b:b + 1])
                nc.sync.dma_start(out=out_ctx.ap()[b:b + 1, :], in_=fctx)

            # Sequential blocks: each block's exp/DVE chain overlaps the next
            # block's projection matmuls, so only the final block's chain is
            # exposed at the tail.
            for blk in range(NBLK):
                if blk == 0:
                    qk_t, v_t = None, v_first  # qk lives in qk_f0/qk_f1 halves
                else:
                    qk_t = qk_pool.tile([128, KC, SB], BF16, name="qk_t")
                    nc.sync.dma_start(
                        out=qk_t, in_=qkT.ap()[:, :, blk * SB:(blk + 1) * SB]
                    )
                    v_t = v_pool.tile([128, DC, SB], BF16, name="v_t")
                    nc.sync.dma_start(
                        out=v_t, in_=valT.ap()[:, :, blk * SB:(blk + 1) * SB]
                    )
                if blk == 0:
                    def rhs_of(kc):
                        return qk_q[kc // Q][:, kc % Q, :]
                    v_t = v_first
                else:
                    def rhs_of(kc, qk_t=qk_t):
                        return qk_t[:, kc, :]

                tT = t_pool.tile([128, AC, SB], BF16, name="tT")
                for ap_ in range(AC // 2):
                    z_ps = zt_pool.tile([128, 2, SB], F32, name="z_ps")
                    for half in range(2):
                        a = 2 * ap_ + half
                        for kc in range(KC):
                            nc.tensor.matmul(
                                z_ps[:, half, :],
                                lhsT=wc_lhsT(kc, a),
                                rhs=rhs_of(kc),
                                start=(kc == 0),
                                stop=(kc == KC - 1),
                            )
                    nc.scalar.activation(
                        out=tT[:, 2 * ap_:2 * ap_ + 2, :], in_=z_ps, func=AF.Tanh
                    )

                emit_scores_exp_reduce(blk, tT, v_t)
                if blk % BLKB == BLKB - 1:
                    emit_batch_tail(blk // BLKB)

    nc.compile()
    return nc


def _get_nc():
    if "nc" not in _CACHE:
        _CACHE["nc"] = _build()
    return _CACHE["nc"]


def _prep_core(q2, k2, v2):
    """Host-side layout prep for one core's shard (free: not on-device time)."""
    xcatT = np.concatenate([q2.T, k2.T], 0)  # [2D, SL]
    qkT = np.ascontiguousarray(
        xcatT.reshape(KC, 128, SL).transpose(1, 0, 2)
    ).astype(NPBF16)
    valT = np.ascontiguousarray(
        v2.T.reshape(DC, 128, SL).transpose(1, 0, 2)
    ).astype(NPBF16)
    return qkT, valT


def _prep_weights(Wq, Wk, Wv, Ws):
    Wcat = np.concatenate([Wq, Wk], 0)  # [2D, A]
    wcat_h = np.ascontiguousarray(
        Wcat.reshape(KC, 128, A).transpose(1, 0, 2)
    ).astype(NPBF16)
    # Ws replicated across 128 PE columns: the scores matmul then broadcasts
    # the score row to every output partition.
    ws_pa = Ws[:, 0].reshape(AC, 128).transpose(1, 0)  # [128, AC]
    wsr_h = np.ascontiguousarray(
        np.repeat(ws_pa[:, :, None], 128, axis=2)
    ).astype(NPBF16)
    wvp_h = np.ascontiguousarray(
        Wv.reshape(DC, 128, A).transpose(1, 0, 2)
    ).astype(NPBF16)
    return wcat_h, wsr_h, wvp_h


def build_in_maps(query, key_, value, Wq, Wk, Wv, Ws):
    query = np.asarray(query, dtype=np.float32)
    key_ = np.asarray(key_, dtype=np.float32)
    value = np.asarray(value, dtype=np.float32)
    wcat_h, wsr_h, wvp_h = _prep_weights(
        np.asarray(Wq, dtype=np.float32),
        np.asarray(Wk, dtype=np.float32),
        np.asarray(Wv, dtype=np.float32),
        np.asarray(Ws, dtype=np.float32),
    )
    in_maps = []
    for c in range(NCORES):
        q2 = query[c * BPC:(c + 1) * BPC].reshape(SL, D)
        k2 = key_[c * BPC:(c + 1) * BPC].reshape(SL, D)
        v2 = value[c * BPC:(c + 1) * BPC].reshape(SL, D)
        qkT_h, valT_h = _prep_core(q2, k2, v2)
        in_maps.append(
            {"qkT": qkT_h, "valT": valT_h, "wcat": wcat_h, "wsr": wsr_h,
             "wvp": wvp_h}
        )
    return in_maps


def kernel(query, key_, value, Wq, Wk, Wv, Ws):
    nc = _get_nc()
    in_maps = build_in_maps(query, key_, value, Wq, Wk, Wv, Ws)

    res = bass_utils.run_bass_kernel_spmd(
        nc, in_maps, core_ids=list(range(NCORES))
    )

    ctx = np.concatenate(
        [np.asarray(r["out_ctx"], dtype=np.float32) for r in res.results], 0
    )
    attw = np.concatenate(
        [np.asarray(r["out_w"], dtype=np.float32) for r in res.results], 0
    )[..., None]
    return ctx, attw
